# revision 5
# baseline (speedup 1.0000x reference)
"""ConVIRT loss (NT-Xent both directions) on 8 Trainium2 NeuronCores.

Strategy: shard img rows across 8 cores; each core computes its
[8192 (text j) x 1024 (img i)] slab of the similarity matrix in fp8
DoubleRow matmuls (2x PE throughput).  The img operand carries its
cosine norm (x32 fp8 range scale); the text norm rides the per-partition
ACT exp scale.

Feeds per core (host-prepped, bf16):
  z_img      [1024, 512]     core's img block, natural
  z_textT    [512, 8192]     full text, TRANSPOSED (DMA-cast to fp8
                             matmul operand; no on-device transposes)
  z_text_q   [16, 128, 2048] full text rows quad-packed
                             [q][p][s*512+d] = text[q*512+s*128+p, d]
                             (row sumsq stream -> tscale)
  z_text_blk [1024, 512]     core's text block rows (diag + norms)

Main loop per jt: 4 fp8 DR matmuls fill a 2-bank psum tile [j=128,
i=1024]; ONE ACT exp (N=1024 amortizes the 352-cycle ACT fixed cost)
writes an fp8 e-tile slot and accumulates colsum; per jt-pair two fp8
DR ones-matmuls accumulate rowsum in psum.  All input DMAs ride the
idle sync-engine HWDGE queue except the fp8 DMA-casts (gpsimd swdge).
Diag via affine_mul_reduce dot of normalized img (bf16) x raw text.
Host combines: loss = a*mean(log(rs)-d) + (1-a)*mean(log(cs)-d).
"""

import math
import numpy as np
import ml_dtypes

import concourse.bacc as bacc
import concourse.tile as tile
import concourse.mybir as mybir
from concourse.bass_utils import run_bass_kernel_spmd

N, D = 8192, 512
CORES = 8
BLK = N // CORES          # 1024 img rows per core
NT = N // 128             # 64 text j-tiles
NTI = BLK // 128          # 8 img tiles per core
KC = D // 128             # 4 contraction chunks
KP = KC // 2              # 2 fp8 DoubleRow chunk-pairs
IC = BLK // 512           # 2 rowsum chunks of 512
NPAIR = NT // 2           # 32 jt-pairs for the rowsum ones-matmul
NQ = N // 512             # 16 natural-text quads
TPC = 2048                # textT DMA piece columns
FS = 32.0                 # fp8 range scale on the img operand
TEMP, ALPHA, EPS = 0.1, 0.75, 1e-8

f32 = mybir.dt.float32
bf16 = mybir.dt.bfloat16
fp8 = mybir.dt.float8e4
AF = mybir.ActivationFunctionType
ALU = mybir.AluOpType
AX = mybir.AxisListType
PM = mybir.MatmulPerfMode

_CACHE = {}
_IDENT = np.eye(128).astype(ml_dtypes.bfloat16)


def _norm_finish(nc, pool, ss_ap, out_ap, bias_ap):
    """out = exp(-0.5*ln(max(ss, eps)) + bias) = e^bias / sqrt(ss).

    Stays inside the natural_log_exp_and_others ACT table set (no table
    switches vs the main-loop Exp).  bias_ap: [128,1] f32.
    """
    n = ss_ap.shape[-1]
    t0 = pool.tile([128, n], f32, tag="nf0")
    t1 = pool.tile([128, n], f32, tag="nf1")
    nc.vector.tensor_scalar_max(t0[:], ss_ap, EPS * EPS)
    nc.scalar.activation(t1[:], t0[:], AF.Ln)
    nc.scalar.activation(out_ap, t1[:], AF.Exp, scale=-0.5, bias=bias_ap)


def _build():
    nc = bacc.Bacc("TRN2", target_bir_lowering=False, debug=False)

    z_img = nc.dram_tensor("z_img", [BLK, D], bf16, kind="ExternalInput")
    z_textT = nc.dram_tensor("z_textT", [D, N], bf16, kind="ExternalInput")
    z_text_q = nc.dram_tensor("z_text_q", [NQ, 128, 2048], bf16, kind="ExternalInput")
    z_text_blk = nc.dram_tensor("z_text_blk", [BLK, D], bf16, kind="ExternalInput")
    ident = nc.dram_tensor("ident", [128, 128], bf16, kind="ExternalInput")
    out_rowsum = nc.dram_tensor("out_rowsum", [1, BLK], f32, kind="ExternalOutput")
    out_colsum = nc.dram_tensor("out_colsum", [128, NT], f32, kind="ExternalOutput")
    out_diag = nc.dram_tensor("out_diag", [128, NTI], f32, kind="ExternalOutput")

    with tile.TileContext(nc) as tc:
        with (
            tc.tile_pool(name="pers", bufs=1) as pers,
            tc.tile_pool(name="imgraw", bufs=NTI) as imgpool,
            tc.tile_pool(name="ld", bufs=8) as ldpool,
            tc.tile_pool(name="ldq", bufs=4) as ldqpool,
            tc.tile_pool(name="sq", bufs=3) as sqpool,
            tc.tile_pool(name="nf", bufs=2) as nfpool,
            tc.tile_pool(name="e2", bufs=2) as e2pool,
            tc.tile_pool(name="ps", bufs=2, space="PSUM") as pspool,
            tc.tile_pool(name="psr", bufs=1, space="PSUM") as psrpool,
            tc.tile_pool(name="pst", bufs=2, space="PSUM") as pstpool,
        ):
            identSB = pers.tile([128, 128], bf16, tag="identSB")
            nc.sync.dma_start(identSB[:], ident[:])

            # DR weights need the 2-slot dim stride %16 B: pad cols to 16
            ones8 = pers.tile([128, 2, 16], fp8, tag="ones8")
            nc.vector.memset(ones8[:], 1.0)
            # iscale = FS / r_i  (bias = ln FS)
            biasI = pers.tile([128, 1], f32, tag="biasI")
            nc.vector.memset(biasI[:], math.log(FS))
            # tscale = 1 / (FS * T * t_j)  (bias = -ln(FS T))
            biasT = pers.tile([128, 1], f32, tag="biasT")
            nc.vector.memset(biasT[:], -math.log(FS * TEMP))

            textT8 = [pers.tile([128, 2, N], fp8, tag=f"textT8_{p}", name=f"textT8_{p}")
                      for p in range(KP)]
            imgT8 = [pers.tile([128, 2, BLK], fp8, tag=f"imgT8_{p}", name=f"imgT8_{p}")
                     for p in range(KP)]
            img_nb = pers.tile([128, NTI, D], bf16, tag="img_nb")
            tblk = pers.tile([128, NTI, D], bf16, tag="tblk")
            tss = pers.tile([128, NT], f32, tag="tss")
            tscale = pers.tile([128, NT], f32, tag="tscale")
            iss = pers.tile([128, NTI], f32, tag="iss")
            iscale = pers.tile([128, NTI], f32, tag="iscale")
            bss = pers.tile([128, NTI], f32, tag="bss")
            bscale = pers.tile([128, NTI], f32, tag="bscale")
            dots = pers.tile([128, NTI], f32, tag="dots")
            diagb = pers.tile([128, NTI], f32, tag="diagb")
            csacc = pers.tile([128, NT], f32, tag="csacc")
            rs = pers.tile([1, BLK], f32, tag="rs")

            # ---- input DMA triggers, in need-order.
            # img tiles (sync HWDGE): gate the whole PE stream.
            imgraw = []
            for t in range(NTI):
                r = imgpool.tile([128, D], bf16, tag="imgraw", name=f"imgraw{t}")
                nc.sync.dma_start(r[:], z_img[t * 128:(t + 1) * 128, :])
                imgraw.append(r)
            # textT fp8 DMA-casts (gpsimd swdge), piece 0 first (jt 0..15),
            # interleaved with the quad stream feeding tscale.
            qtiles = [None] * NQ

            def load_quad(q):
                qt = ldqpool.tile([128, 4, 512], bf16, tag="ldq", name=f"ldq{q}")
                nc.sync.dma_start(qt[:], z_text_q[q])
                qtiles[q] = qt

            def load_piece(pc):
                cs = slice(pc * TPC, (pc + 1) * TPC)
                for k in range(KC):
                    nc.gpsimd.dma_start(textT8[k // 2][:, k % 2, cs],
                                        z_textT[k * 128:(k + 1) * 128, cs])

            load_quad(0)
            load_piece(0)
            for q in range(1, 4):
                load_quad(q)
            load_piece(1)
            # text blk rows for diag (sync queue; not urgent)
            blkraw = []
            for t in range(NTI):
                r = ldpool.tile([128, D], bf16, tag="braw", name=f"braw{t}")
                nc.sync.dma_start(r[:], z_text_blk[t * 128:(t + 1) * 128, :])
                blkraw.append(r)
            for pc in range(2, N // TPC):
                load_piece(pc)
            for q in range(4, NQ):
                load_quad(q)

            # ---- tscale quad 0 first (ACT needs it right after PE's jt=0)
            def quad_sumsq(q):
                qt = qtiles[q]
                for s in range(4):
                    sq = sqpool.tile([128, 512], bf16, tag="sq")
                    nc.vector.affine_mul_reduce(
                        sq[:], tss[:, 4 * q + s:4 * q + s + 1],
                        qt[:, s, :], qt[:, s, :], 1.0, 0.0)
                _norm_finish(nc, nfpool, tss[:, 4 * q:4 * q + 4],
                             tscale[:, 4 * q:4 * q + 4], biasT[:])

            quad_sumsq(0)

            # ---- img: sumsq, normalize (FS/r_i folded, bf16), transpose
            for t in range(NTI):
                sq = sqpool.tile([128, D], bf16, tag="sq")
                nc.vector.affine_mul_reduce(
                    sq[:], iss[:, t:t + 1], imgraw[t][:], imgraw[t][:], 1.0, 0.0)
            _norm_finish(nc, nfpool, iss[:], iscale[:], biasI[:])
            for t in range(NTI):
                nc.vector.tensor_scalar(
                    img_nb[:, t, :], imgraw[t][:], iscale[:, t:t + 1], None, ALU.mult)
                for k in range(KC):
                    pst = pstpool.tile([128, 128], bf16, tag="pst")
                    nc.tensor.transpose(pst[:], img_nb[:, t, k * 128:(k + 1) * 128], identSB[:])
                    nc.vector.tensor_copy(imgT8[k // 2][:, k % 2, t * 128:(t + 1) * 128], pst[:])

            for q in range(1, 4):
                quad_sumsq(q)

            # ---- text block rows: sumsq + diag dot
            for t in range(NTI):
                sq = sqpool.tile([128, D], bf16, tag="sq")
                nc.vector.affine_mul_reduce(
                    sq[:], bss[:, t:t + 1], blkraw[t][:], blkraw[t][:], 1.0, 0.0)
                nc.vector.tensor_copy(tblk[:, t, :], blkraw[t][:])
            _norm_finish(nc, nfpool, bss[:], bscale[:], biasT[:])
            # diag_r = dot(FS*img_n[r], text_raw[r]) / (FS*T*t_r) = cos/T
            for t in range(NTI):
                sq = sqpool.tile([128, D], bf16, tag="sq")
                nc.vector.affine_mul_reduce(
                    sq[:], dots[:, t:t + 1], img_nb[:, t, :], tblk[:, t, :], 1.0, 0.0)
            nc.vector.tensor_tensor(diagb[:], dots[:], bscale[:], op=ALU.mult)
            nc.sync.dma_start(out_diag[:], diagb[:])

            for q in range(4, NQ):
                quad_sumsq(q)

            # ---- main loop: 2-bank psum tile [j=128, i=1024] per jt
            psrow = [psrpool.tile([1, 512], f32, tag=f"psr{ic}", name=f"psr{ic}")
                     for ic in range(IC)]
            e2t = None
            for jt in range(NT):
                pr, slot = jt // 2, jt % 2
                if slot == 0:
                    e2t = e2pool.tile([128, 2, 1024], fp8, tag="e2", name=f"e2_{pr}")
                ps = pspool.tile([128, 1024], f32, tag="ps")
                for ic in range(IC):
                    for p in range(KP):
                        nc.tensor.matmul(
                            ps[:, ic * 512:(ic + 1) * 512],
                            textT8[p][:, :, jt * 128:(jt + 1) * 128],
                            imgT8[p][:, :, ic * 512:(ic + 1) * 512],
                            start=(p == 0), stop=(p == KP - 1),
                            perf_mode=PM.DoubleRow)
                nc.scalar.activation(
                    e2t[:, slot, :], ps[:], AF.Exp,
                    scale=tscale[:, jt:jt + 1],
                    accum_out=csacc[:, jt:jt + 1])
                if slot == 1:
                    for ic in range(IC):
                        nc.tensor.matmul(
                            psrow[ic][:], ones8[:, :, 0:1],
                            e2t[:, :, ic * 512:(ic + 1) * 512],
                            start=(pr == 0), stop=(pr == NPAIR - 1),
                            perf_mode=PM.DoubleRow,
                            skip_group_check=True)

            # ---- finish: rowsum psum -> sbuf -> dram; colsum direct
            for ic in range(IC):
                nc.vector.tensor_copy(rs[:, ic * 512:(ic + 1) * 512], psrow[ic][:])
            nc.sync.dma_start(out_rowsum[:], rs[:])
            nc.sync.dma_start(out_colsum[:], csacc[:])

    nc.compile()
    return nc


def get_program():
    if "nc" not in _CACHE:
        _CACHE["nc"] = _build()
    return _CACHE["nc"]


def make_in_maps(z_img, z_text):
    zi = np.asarray(z_img, dtype=np.float32).astype(ml_dtypes.bfloat16)
    zt = np.asarray(z_text, dtype=np.float32).astype(ml_dtypes.bfloat16)
    ztT = np.ascontiguousarray(zt.T)
    ztq = np.ascontiguousarray(
        zt.reshape(NQ, 4, 128, D).transpose(0, 2, 1, 3)).reshape(NQ, 128, 2048)
    maps = []
    for c in range(CORES):
        blk = slice(c * BLK, (c + 1) * BLK)
        maps.append({
            "z_img": zi[blk],
            "z_textT": ztT,
            "z_text_q": ztq,
            "z_text_blk": zt[blk],
            "ident": _IDENT,
        })
    return maps


def combine(results):
    rows = np.concatenate([r["out_rowsum"][0] for r in results])          # [8192]
    cols = np.zeros((128, NT), np.float64)
    for r in results:
        cols += r["out_colsum"]
    colsum = cols.T.reshape(-1)                                           # j = jt*128+p
    diag = np.concatenate([r["out_diag"].T.reshape(-1) for r in results])
    loss_a = np.mean(np.log(rows.astype(np.float64)) - diag)
    loss_b = np.mean(np.log(colsum) - diag)
    return np.float32(ALPHA * loss_a + (1.0 - ALPHA) * loss_b)


def _run_sim(nc, maps):
    from concourse.bass_interp import CoreSim
    outs = []
    for m in maps:
        sim = CoreSim(nc, trace=False)
        for k, v in m.items():
            sim.tensor(k)[:] = v
        sim.simulate()
        outs.append({n: np.array(sim.tensor(n))
                     for n in ("out_rowsum", "out_colsum", "out_diag")})
    return outs


def kernel(z_img, z_text):
    nc = get_program()
    maps = make_in_maps(z_img, z_text)
    for _ in range(3):
        try:
            res = run_bass_kernel_spmd(nc, maps, list(range(CORES))).results
            return combine(res)
        except Exception:  # transient device hiccups: retry, then sim
            pass
    res = _run_sim(nc, maps)
    return combine(res)


if __name__ == "__main__":
    rng = np.random.default_rng(0)
    out = kernel(rng.standard_normal((N, D), dtype=np.float32),
                 rng.standard_normal((N, D), dtype=np.float32))
    print("loss:", out)


# revision 7
# speedup vs baseline: 1.3595x; 1.3595x over previous
"""ConVIRT loss (NT-Xent both directions) on 8 Trainium2 NeuronCores.

Strategy: shard img rows across 8 cores; each core computes its
[8192 (text j) x 1024 (img i)] slab of the similarity matrix in fp8
DoubleRow matmuls (2x PE throughput).  The img operand carries its
cosine norm (x32 fp8 range scale); the text norm rides the per-partition
ACT exp scale.

Feeds per core (host-prepped, bf16; packing is pure layout):
  z_img_q  [128, 8, 512]     img block rows slot-packed (one DMA)
  z_textT  [512, 8192]       full text TRANSPOSED -> fp8 matmul operand
                             via gpsimd DMA-cast (no device transposes)
  z_text_q [8, 128, 8, 512]  full text rows oct-packed (row sumsq)
  z_blk_q  [128, 8, 512]     core's text block rows (diag + norms)

Main loop per jt: 4 fp8 DR matmuls fill a 2-bank psum tile [j=128,
i=1024]; ONE ACT exp (N=1024 amortizes the 352-cycle ACT fixed cost)
writes an fp8 e-tile slot and accumulates colsum; per jt-pair two fp8
DR ones-matmuls accumulate rowsum in psum.  All norms (1/sqrt) are
computed on DVE with the bit-trick + 2 Newton steps so the ACT table
set never leaves Exp (table reloads cost 1.3us each).  Input DMAs ride
the sync-engine HWDGE queue; fp8 casts ride the gpsimd swdge.
Host combines: loss = a*mean(log(rs)-d) + (1-a)*mean(log(cs)-d).
"""

import math
import numpy as np
import ml_dtypes

import concourse.bacc as bacc
import concourse.tile as tile
import concourse.mybir as mybir
from concourse.bass_utils import run_bass_kernel_spmd

N, D = 8192, 512
CORES = 8
BLK = N // CORES          # 1024 img rows per core
NT = N // 128             # 64 text j-tiles
NTI = BLK // 128          # 8 img tiles per core
KC = D // 128             # 4 contraction chunks
KP = KC // 2              # 2 fp8 DoubleRow chunk-pairs
IC = BLK // 512           # 2 rowsum chunks of 512
NPAIR = NT // 2           # 32 jt-pairs for the rowsum ones-matmul
NO = 8                    # text row octs
TPC = 2048                # textT DMA-cast piece columns
FS = 32.0                 # fp8 range scale on the img operand
TEMP, ALPHA = 0.1, 0.75

f32 = mybir.dt.float32
bf16 = mybir.dt.bfloat16
fp8 = mybir.dt.float8e4
i32 = mybir.dt.int32
AF = mybir.ActivationFunctionType
ALU = mybir.AluOpType
PM = mybir.MatmulPerfMode

_CACHE = {}


def _rsqrt(nc, pool, ss_ap, out_ap, mult):
    """out = mult / sqrt(ss) on DVE: Quake bit-trick + 2 Newton steps.

    Avoids ACT Ln/Sqrt so the exp table set stays resident.  ~1e-5 rel.
    """
    n = ss_ap.shape[-1]
    y = pool.tile([128, n], f32, tag="rq_y")
    t = pool.tile([128, n], f32, tag="rq_t")
    # y = bitcast(0x5f3759df - (bitcast_i32(ss) >> 1)) = ~((ss>>1)^-1)+C+1
    nc.vector.tensor_scalar(y[:].bitcast(i32), ss_ap.bitcast(i32),
                            1, -1, ALU.logical_shift_right,
                            ALU.bitwise_xor)
    nc.vector.tensor_scalar_add(y[:].bitcast(i32), y[:].bitcast(i32),
                                0x5F3759E0)
    # Newton 1: y *= 1.5 - 0.5*ss*y^2
    nc.vector.tensor_tensor(t[:], y[:], y[:], op=ALU.mult)
    nc.vector.tensor_tensor(t[:], t[:], ss_ap, op=ALU.mult)
    nc.vector.tensor_scalar(t[:], t[:], -0.5, 1.5, ALU.mult, ALU.add)
    nc.vector.tensor_tensor(y[:], y[:], t[:], op=ALU.mult)
    # Newton 2, final mult folded in: y *= c*1.5 - c*0.5*ss*y^2
    nc.vector.tensor_tensor(t[:], y[:], y[:], op=ALU.mult)
    nc.vector.tensor_tensor(t[:], t[:], ss_ap, op=ALU.mult)
    nc.vector.tensor_scalar(t[:], t[:], -0.5 * mult, 1.5 * mult,
                            ALU.mult, ALU.add)
    nc.vector.tensor_tensor(out_ap, y[:], t[:], op=ALU.mult)


def _build():
    nc = bacc.Bacc("TRN2", target_bir_lowering=False, debug=False)

    z_img_q = nc.dram_tensor("z_img_q", [128, NTI, D], bf16, kind="ExternalInput")
    z_textT = nc.dram_tensor("z_textT", [D, N], bf16, kind="ExternalInput")
    z_text_q = nc.dram_tensor("z_text_q", [NO, 128, 8, D], bf16, kind="ExternalInput")
    z_blk_q = nc.dram_tensor("z_blk_q", [128, NTI, D], bf16, kind="ExternalInput")
    ident = nc.dram_tensor("ident", [128, 128], bf16, kind="ExternalInput")
    out_rowsum = nc.dram_tensor("out_rowsum", [1, BLK], f32, kind="ExternalOutput")
    out_colsum = nc.dram_tensor("out_colsum", [128, NT], f32, kind="ExternalOutput")
    out_diag = nc.dram_tensor("out_diag", [128, NTI], f32, kind="ExternalOutput")

    with tile.TileContext(nc) as tc:
        with (
            tc.tile_pool(name="pers", bufs=1) as pers,
            tc.tile_pool(name="ldq", bufs=3) as ldqpool,
            tc.tile_pool(name="sq", bufs=3) as sqpool,
            tc.tile_pool(name="rq", bufs=2) as rqpool,
            tc.tile_pool(name="e2", bufs=2) as e2pool,
            tc.tile_pool(name="ps", bufs=2, space="PSUM") as pspool,
            tc.tile_pool(name="psr", bufs=1, space="PSUM") as psrpool,
            tc.tile_pool(name="pst", bufs=2, space="PSUM") as pstpool,
        ):
            imgq = pers.tile([128, NTI, D], bf16, tag="imgq")
            identSB = pers.tile([128, 128], bf16, tag="identSB")
            tblk = pers.tile([128, NTI, D], bf16, tag="tblk")
            textT8 = [pers.tile([128, 2, N], fp8, tag=f"textT8_{p}", name=f"textT8_{p}")
                      for p in range(KP)]
            imgT8 = [pers.tile([128, 2, BLK], fp8, tag=f"imgT8_{p}", name=f"imgT8_{p}")
                     for p in range(KP)]
            img_nb = pers.tile([128, NTI, D], bf16, tag="img_nb")
            tss = pers.tile([128, NT], f32, tag="tss")
            tscale = pers.tile([128, NT], f32, tag="tscale")
            iss = pers.tile([128, NTI], f32, tag="iss")
            iscale = pers.tile([128, NTI], f32, tag="iscale")
            bss = pers.tile([128, NTI], f32, tag="bss")
            bscale = pers.tile([128, NTI], f32, tag="bscale")
            dots = pers.tile([128, NTI], f32, tag="dots")
            diagb = pers.tile([128, NTI], f32, tag="diagb")
            csacc = pers.tile([128, NT], f32, tag="csacc")
            rs = pers.tile([1, BLK], f32, tag="rs")

            # DR weights need the 2-slot dim stride %16 B: pad cols to 16
            ones8 = pers.tile([128, 2, 16], fp8, tag="ones8")
            nc.vector.memset(ones8[:], 1.0)

            # ---- input DMA triggers in need-order.
            # sync HWDGE: img gates PE; oct0 gates ACT's first exp scale.
            nc.sync.dma_start(imgq[:], z_img_q[:])
            octs = []
            for q in range(NO):
                qt = ldqpool.tile([128, 8, D], bf16, tag="ldq", name=f"oct{q}")
                nc.sync.dma_start(qt[:], z_text_q[q])
                octs.append(qt)
                if q == 0:
                    nc.sync.dma_start(identSB[:], ident[:])
                    nc.sync.dma_start(tblk[:], z_blk_q[:])
            # gpsimd swdge fp8 DMA-casts, first piece (jt 0..15) first
            for pc in range(N // TPC):
                cs = slice(pc * TPC, (pc + 1) * TPC)
                for k in range(KC):
                    nc.gpsimd.dma_start(textT8[k // 2][:, k % 2, cs],
                                        z_textT[k * 128:(k + 1) * 128, cs])

            # ---- img: sumsq, normalize (FS/r_i folded, bf16), transpose
            for t in range(NTI):
                sq = sqpool.tile([128, D], bf16, tag="sq")
                nc.vector.affine_mul_reduce(
                    sq[:], iss[:, t:t + 1], imgq[:, t, :], imgq[:, t, :], 1.0, 0.0)
            _rsqrt(nc, rqpool, iss[:], iscale[:], FS)
            for t in range(NTI):
                nc.vector.tensor_scalar(
                    img_nb[:, t, :], imgq[:, t, :], iscale[:, t:t + 1], None, ALU.mult)
                for p in range(KP):
                    pst = pstpool.tile([128, 2, 128], bf16, tag="pst")
                    for c in range(2):
                        nc.tensor.transpose(
                            pst[:, c, :],
                            img_nb[:, t, (2 * p + c) * 128:(2 * p + c + 1) * 128],
                            identSB[:])
                    nc.vector.tensor_copy(imgT8[p][:, :, t * 128:(t + 1) * 128], pst[:])

            # ---- text row sumsq -> tscale = rsqrt(ss)/(FS*T), per oct
            for q in range(NO):
                for s in range(8):
                    sq = sqpool.tile([128, D], bf16, tag="sq")
                    nc.vector.affine_mul_reduce(
                        sq[:], tss[:, 8 * q + s:8 * q + s + 1],
                        octs[q][:, s, :], octs[q][:, s, :], 1.0, 0.0)
                _rsqrt(nc, rqpool, tss[:, 8 * q:8 * q + 8],
                       tscale[:, 8 * q:8 * q + 8], 1.0 / (FS * TEMP))

            # ---- text block rows: sumsq (scalar_tensor_tensor) + diag dot
            for t in range(NTI):
                sq = sqpool.tile([128, D], bf16, tag="sq")
                nc.vector.scalar_tensor_tensor(
                    out=sq[:], in0=tblk[:, t, :], scalar=1.0, in1=tblk[:, t, :],
                    op0=ALU.mult, op1=ALU.mult, accum_out=bss[:, t:t + 1])
            _rsqrt(nc, rqpool, bss[:], bscale[:], 1.0 / (FS * TEMP))
            # diag_r = dot(FS*img_n[r], text_raw[r]) / (FS*T*t_r) = cos/T
            for t in range(NTI):
                sq = sqpool.tile([128, D], bf16, tag="sq")
                nc.vector.affine_mul_reduce(
                    sq[:], dots[:, t:t + 1], img_nb[:, t, :], tblk[:, t, :], 1.0, 0.0)
            nc.vector.tensor_tensor(diagb[:], dots[:], bscale[:], op=ALU.mult)

            # ---- main loop: 2-bank psum tile [j=128, i=1024] per jt
            psrow = [psrpool.tile([1, 512], f32, tag=f"psr{ic}", name=f"psr{ic}")
                     for ic in range(IC)]
            e2t = None
            for jt in range(NT):
                pr, slot = jt // 2, jt % 2
                if slot == 0:
                    e2t = e2pool.tile([128, 2, 1024], fp8, tag="e2", name=f"e2_{pr}")
                ps = pspool.tile([128, 1024], f32, tag="ps")
                for ic in range(IC):
                    for p in range(KP):
                        nc.tensor.matmul(
                            ps[:, ic * 512:(ic + 1) * 512],
                            textT8[p][:, :, jt * 128:(jt + 1) * 128],
                            imgT8[p][:, :, ic * 512:(ic + 1) * 512],
                            start=(p == 0), stop=(p == KP - 1),
                            perf_mode=PM.DoubleRow)
                nc.scalar.activation(
                    e2t[:, slot, :], ps[:], AF.Exp,
                    scale=tscale[:, jt:jt + 1],
                    accum_out=csacc[:, jt:jt + 1])
                if slot == 1:
                    for ic in range(IC):
                        nc.tensor.matmul(
                            psrow[ic][:], ones8[:, :, 0:1],
                            e2t[:, :, ic * 512:(ic + 1) * 512],
                            start=(pr == 0), stop=(pr == NPAIR - 1),
                            perf_mode=PM.DoubleRow,
                            skip_group_check=True)

            # ---- finish: rowsum psum -> sbuf -> dram; colsum/diag direct
            for ic in range(IC):
                nc.vector.tensor_copy(rs[:, ic * 512:(ic + 1) * 512], psrow[ic][:])
            nc.sync.dma_start(out_diag[:], diagb[:])
            nc.sync.dma_start(out_rowsum[:], rs[:])
            nc.sync.dma_start(out_colsum[:], csacc[:])

    nc.compile()
    return nc


def get_program():
    if "nc" not in _CACHE:
        _CACHE["nc"] = _build()
    return _CACHE["nc"]


def _quad_pack(a):  # [R, 512] -> [128, R//128, 512], rows r = s*128+p
    return np.ascontiguousarray(a.reshape(-1, 128, D).transpose(1, 0, 2))


def make_in_maps(z_img, z_text):
    zi = np.asarray(z_img, dtype=np.float32).astype(ml_dtypes.bfloat16)
    zt = np.asarray(z_text, dtype=np.float32).astype(ml_dtypes.bfloat16)
    ztT = np.ascontiguousarray(zt.T)
    ztq = np.ascontiguousarray(
        zt.reshape(NO, 8, 128, D).transpose(0, 2, 1, 3))
    maps = []
    for c in range(CORES):
        blk = slice(c * BLK, (c + 1) * BLK)
        maps.append({
            "z_img_q": _quad_pack(zi[blk]),
            "z_textT": ztT,
            "z_text_q": ztq,
            "z_blk_q": _quad_pack(zt[blk]),
            "ident": np.eye(128).astype(ml_dtypes.bfloat16),
        })
    return maps


def combine(results):
    rows = np.concatenate([r["out_rowsum"][0] for r in results])          # [8192]
    cols = np.zeros((128, NT), np.float64)
    for r in results:
        cols += r["out_colsum"]
    colsum = cols.T.reshape(-1)                                           # j = jt*128+p
    diag = np.concatenate([r["out_diag"].T.reshape(-1) for r in results])
    loss_a = np.mean(np.log(rows.astype(np.float64)) - diag)
    loss_b = np.mean(np.log(colsum) - diag)
    return np.float32(ALPHA * loss_a + (1.0 - ALPHA) * loss_b)


def _run_sim(nc, maps):
    from concourse.bass_interp import CoreSim
    outs = []
    for m in maps:
        sim = CoreSim(nc, trace=False)
        for k, v in m.items():
            sim.tensor(k)[:] = v
        sim.simulate()
        outs.append({n: np.array(sim.tensor(n))
                     for n in ("out_rowsum", "out_colsum", "out_diag")})
    return outs


def kernel(z_img, z_text):
    nc = get_program()
    maps = make_in_maps(z_img, z_text)
    for _ in range(3):
        try:
            res = run_bass_kernel_spmd(nc, maps, list(range(CORES))).results
            return combine(res)
        except Exception:  # transient device hiccups: retry, then sim
            pass
    res = _run_sim(nc, maps)
    return combine(res)


if __name__ == "__main__":
    rng = np.random.default_rng(0)
    out = kernel(rng.standard_normal((N, D), dtype=np.float32),
                 rng.standard_normal((N, D), dtype=np.float32))
    print("loss:", out)


# revision 8
# speedup vs baseline: 1.4052x; 1.0336x over previous
"""ConVIRT loss (NT-Xent both directions) on 8 Trainium2 NeuronCores.

Strategy: shard img rows across 8 cores; each core computes its
[8192 (text j) x 1024 (img i)] slab of the similarity matrix in fp8
DoubleRow matmuls (2x PE throughput).  The img operand carries its
cosine norm (x32 fp8 range scale); the text norm rides the per-partition
ACT exp scale.

Feeds per core (host-prepped, bf16; packing is pure layout):
  z_img_q  [128, 8, 512]     img block rows slot-packed (one DMA)
  z_textT  [512, 8192]       full text TRANSPOSED -> fp8 matmul operand
                             via gpsimd DMA-cast (no device transposes)
  z_text_q [8, 128, 8, 512]  full text rows oct-packed (row sumsq)
  z_blk_q  [128, 8, 512]     core's text block rows (diag + norms)

Main loop per jt: 4 fp8 DR matmuls fill a 2-bank psum tile [j=128,
i=1024]; ONE ACT exp (N=1024 amortizes the 352-cycle ACT fixed cost)
writes an fp8 e-tile slot and accumulates colsum; per jt-pair two fp8
DR ones-matmuls accumulate rowsum in psum.  All norms (1/sqrt) are
computed on DVE with the bit-trick + 2 Newton steps so the ACT table
set never leaves Exp (table reloads cost 1.3us each).  Input DMAs ride
the sync-engine HWDGE queue; fp8 casts ride the gpsimd swdge.
Host combines: loss = a*mean(log(rs)-d) + (1-a)*mean(log(cs)-d).
"""

import math
import numpy as np
import ml_dtypes

import concourse.bacc as bacc
import concourse.tile as tile
import concourse.mybir as mybir
from concourse.bass_utils import run_bass_kernel_spmd

N, D = 8192, 512
CORES = 8
BLK = N // CORES          # 1024 img rows per core
NT = N // 128             # 64 text j-tiles
NTI = BLK // 128          # 8 img tiles per core
KC = D // 128             # 4 contraction chunks
KP = KC // 2              # 2 fp8 DoubleRow chunk-pairs
IC = BLK // 512           # 2 rowsum chunks of 512
NPAIR = NT // 2           # 32 jt-pairs for the rowsum ones-matmul
NO = 8                    # text row octs
TPC = 2048                # textT DMA-cast piece columns
FS = 32.0                 # fp8 range scale on the img operand
TEMP, ALPHA = 0.1, 0.75

f32 = mybir.dt.float32
bf16 = mybir.dt.bfloat16
fp8 = mybir.dt.float8e4
i32 = mybir.dt.int32
AF = mybir.ActivationFunctionType
ALU = mybir.AluOpType
PM = mybir.MatmulPerfMode

_CACHE = {}


def _rsqrt(nc, pool, ss_ap, out_ap, mult):
    """out = mult / sqrt(ss) on DVE: Quake bit-trick + 2 Newton steps.

    Avoids ACT Ln/Sqrt so the exp table set stays resident.  ~1e-5 rel.
    """
    n = ss_ap.shape[-1]
    y = pool.tile([128, n], f32, tag="rq_y")
    t = pool.tile([128, n], f32, tag="rq_t")
    # y = bitcast(0x5f3759df - (bitcast_i32(ss) >> 1)) = ~((ss>>1)^-1)+C+1
    nc.vector.tensor_scalar(y[:].bitcast(i32), ss_ap.bitcast(i32),
                            1, -1, ALU.logical_shift_right,
                            ALU.bitwise_xor)
    nc.vector.tensor_scalar_add(y[:].bitcast(i32), y[:].bitcast(i32),
                                0x5F3759E0)
    # Newton 1: y *= 1.5 - 0.5*ss*y^2
    nc.vector.tensor_tensor(t[:], y[:], y[:], op=ALU.mult)
    nc.vector.tensor_tensor(t[:], t[:], ss_ap, op=ALU.mult)
    nc.vector.tensor_scalar(t[:], t[:], -0.5, 1.5, ALU.mult, ALU.add)
    nc.vector.tensor_tensor(y[:], y[:], t[:], op=ALU.mult)
    # Newton 2, final mult folded in: y *= c*1.5 - c*0.5*ss*y^2
    nc.vector.tensor_tensor(t[:], y[:], y[:], op=ALU.mult)
    nc.vector.tensor_tensor(t[:], t[:], ss_ap, op=ALU.mult)
    nc.vector.tensor_scalar(t[:], t[:], -0.5 * mult, 1.5 * mult,
                            ALU.mult, ALU.add)
    nc.vector.tensor_tensor(out_ap, y[:], t[:], op=ALU.mult)


def _build():
    nc = bacc.Bacc("TRN2", target_bir_lowering=False, debug=False)

    z_img_q = nc.dram_tensor("z_img_q", [128, NTI, D], bf16, kind="ExternalInput")
    z_textT = nc.dram_tensor("z_textT", [D, N], bf16, kind="ExternalInput")
    z_text_q = nc.dram_tensor("z_text_q", [NO, 128, 8, D], bf16, kind="ExternalInput")
    z_blk_q = nc.dram_tensor("z_blk_q", [128, NTI, D], bf16, kind="ExternalInput")
    ident = nc.dram_tensor("ident", [128, 128], bf16, kind="ExternalInput")
    out_rowsum = nc.dram_tensor("out_rowsum", [1, BLK], f32, kind="ExternalOutput")
    out_colsum = nc.dram_tensor("out_colsum", [128, NT], f32, kind="ExternalOutput")
    out_diag = nc.dram_tensor("out_diag", [128, NTI], f32, kind="ExternalOutput")

    with tile.TileContext(nc) as tc:
        with (
            tc.tile_pool(name="pers", bufs=1) as pers,
            tc.tile_pool(name="ldq", bufs=3) as ldqpool,
            tc.tile_pool(name="sq", bufs=3) as sqpool,
            tc.tile_pool(name="rq", bufs=2) as rqpool,
            tc.tile_pool(name="e2", bufs=2) as e2pool,
            tc.tile_pool(name="ps", bufs=2, space="PSUM") as pspool,
            tc.tile_pool(name="psr", bufs=1, space="PSUM") as psrpool,
            tc.tile_pool(name="pst", bufs=2, space="PSUM") as pstpool,
        ):
            imgq = pers.tile([128, NTI, D], bf16, tag="imgq")
            identSB = pers.tile([128, 128], bf16, tag="identSB")
            tblk = pers.tile([128, NTI, D], bf16, tag="tblk")
            textT8 = [pers.tile([128, 2, N], fp8, tag=f"textT8_{p}", name=f"textT8_{p}")
                      for p in range(KP)]
            imgT8 = [pers.tile([128, 2, BLK], fp8, tag=f"imgT8_{p}", name=f"imgT8_{p}")
                     for p in range(KP)]
            img_nb = pers.tile([128, NTI, D], bf16, tag="img_nb")
            tss = pers.tile([128, NT], f32, tag="tss")
            tscale_q = [pers.tile([128, 8], f32, tag=f"tscale{q}", name=f"tscale{q}")
                        for q in range(NO)]
            iss = pers.tile([128, NTI], f32, tag="iss")
            iscale = pers.tile([128, NTI], f32, tag="iscale")
            bss = pers.tile([128, NTI], f32, tag="bss")
            bscale = pers.tile([128, NTI], f32, tag="bscale")
            dots = pers.tile([128, NTI], f32, tag="dots")
            diagb = pers.tile([128, NTI], f32, tag="diagb")
            csacc = pers.tile([128, NT], f32, tag="csacc")
            rs = pers.tile([1, BLK], f32, tag="rs")

            # DR weights need the 2-slot dim stride %16 B: pad cols to 16
            ones8 = pers.tile([128, 2, 16], fp8, tag="ones8")
            nc.vector.memset(ones8[:], 1.0)
            # warm up the Exp table set immediately (load costs 1.3us and
            # otherwise lands on the first real exp's critical path)
            wrm = pers.tile([128, 1], f32, tag="wrm")
            nc.vector.memset(wrm[:], 1.0)
            nc.scalar.activation(wrm[:], wrm[:], AF.Exp)
            gate = pers.tile([1, 16], bf16, tag="gate")

            # ---- input DMA triggers in need-order.
            # sync HWDGE: img gates PE; oct0 gates ACT's first exp scale.
            nc.sync.dma_start(imgq[:], z_img_q[:])
            octs = []
            for q in range(NO):
                qt = ldqpool.tile([128, 8, D], bf16, tag="ldq", name=f"oct{q}")
                nc.sync.dma_start(qt[:], z_text_q[q])
                octs.append(qt)
                if q == 0:
                    nc.sync.dma_start(identSB[:], ident[:])
                    nc.sync.dma_start(tblk[:], z_blk_q[:])
            # gpsimd swdge fp8 DMA-casts, first piece (jt 0..15) first.
            # The gate copy stalls the cast stream until the img DMA lands,
            # so the (PE-gating) img feed isn't starved by cast traffic.
            nc.gpsimd.tensor_copy(gate[:], imgq[0:1, 0, 0:16])
            for pc in range(N // TPC):
                cs = slice(pc * TPC, (pc + 1) * TPC)
                for k in range(KC):
                    nc.gpsimd.dma_start(textT8[k // 2][:, k % 2, cs],
                                        z_textT[k * 128:(k + 1) * 128, cs])

            # ---- img: sumsq, normalize (FS/r_i folded, bf16), transpose.
            # high_priority pins this chain ahead of the oct sumsq stream on
            # DVE -- it gates the PE main loop.
            with tc.high_priority():
                for t in range(NTI):
                    sq = sqpool.tile([128, D], bf16, tag="sq")
                    nc.vector.affine_mul_reduce(
                        sq[:], iss[:, t:t + 1], imgq[:, t, :], imgq[:, t, :], 1.0, 0.0)
                _rsqrt(nc, rqpool, iss[:], iscale[:], FS)
                for t in range(NTI):
                    nc.vector.tensor_scalar(
                        img_nb[:, t, :], imgq[:, t, :], iscale[:, t:t + 1], None, ALU.mult)
                    for p in range(KP):
                        pst = pstpool.tile([128, 2, 128], bf16, tag="pst")
                        for c in range(2):
                            nc.tensor.transpose(
                                pst[:, c, :],
                                img_nb[:, t, (2 * p + c) * 128:(2 * p + c + 1) * 128],
                                identSB[:])
                        nc.vector.tensor_copy(imgT8[p][:, :, t * 128:(t + 1) * 128], pst[:])

            # ---- text row sumsq -> tscale = rsqrt(ss)/(FS*T), per oct
            for q in range(NO):
                for s in range(8):
                    sq = sqpool.tile([128, D], bf16, tag="sq")
                    nc.vector.affine_mul_reduce(
                        sq[:], tss[:, 8 * q + s:8 * q + s + 1],
                        octs[q][:, s, :], octs[q][:, s, :], 1.0, 0.0)
                _rsqrt(nc, rqpool, tss[:, 8 * q:8 * q + 8],
                       tscale_q[q][:], 1.0 / (FS * TEMP))

            # ---- text block rows: sumsq (scalar_tensor_tensor) + diag dot
            for t in range(NTI):
                sq = sqpool.tile([128, D], bf16, tag="sq")
                nc.vector.scalar_tensor_tensor(
                    out=sq[:], in0=tblk[:, t, :], scalar=1.0, in1=tblk[:, t, :],
                    op0=ALU.mult, op1=ALU.mult, accum_out=bss[:, t:t + 1])
            _rsqrt(nc, rqpool, bss[:], bscale[:], 1.0 / (FS * TEMP))
            # diag_r = dot(FS*img_n[r], text_raw[r]) / (FS*T*t_r) = cos/T
            for t in range(NTI):
                sq = sqpool.tile([128, D], bf16, tag="sq")
                nc.vector.affine_mul_reduce(
                    sq[:], dots[:, t:t + 1], img_nb[:, t, :], tblk[:, t, :], 1.0, 0.0)
            nc.vector.tensor_tensor(diagb[:], dots[:], bscale[:], op=ALU.mult)

            # ---- main loop: 2-bank psum tile [j=128, i=1024] per jt
            psrow = [psrpool.tile([1, 512], f32, tag=f"psr{ic}", name=f"psr{ic}")
                     for ic in range(IC)]
            e2t = None
            for jt in range(NT):
                pr, slot = jt // 2, jt % 2
                if slot == 0:
                    e2t = e2pool.tile([128, 2, 1024], fp8, tag="e2", name=f"e2_{pr}")
                ps = pspool.tile([128, 1024], f32, tag="ps")
                for ic in range(IC):
                    for p in range(KP):
                        nc.tensor.matmul(
                            ps[:, ic * 512:(ic + 1) * 512],
                            textT8[p][:, :, jt * 128:(jt + 1) * 128],
                            imgT8[p][:, :, ic * 512:(ic + 1) * 512],
                            start=(p == 0), stop=(p == KP - 1),
                            perf_mode=PM.DoubleRow)
                nc.scalar.activation(
                    e2t[:, slot, :], ps[:], AF.Exp,
                    scale=tscale_q[jt // 8][:, jt % 8:jt % 8 + 1],
                    accum_out=csacc[:, jt:jt + 1])
                if slot == 1:
                    for ic in range(IC):
                        nc.tensor.matmul(
                            psrow[ic][:], ones8[:, :, 0:1],
                            e2t[:, :, ic * 512:(ic + 1) * 512],
                            start=(pr == 0), stop=(pr == NPAIR - 1),
                            perf_mode=PM.DoubleRow,
                            skip_group_check=True)

            # ---- finish: rowsum psum -> sbuf -> dram; colsum/diag direct
            for ic in range(IC):
                nc.vector.tensor_copy(rs[:, ic * 512:(ic + 1) * 512], psrow[ic][:])
            nc.sync.dma_start(out_diag[:], diagb[:])
            nc.sync.dma_start(out_rowsum[:], rs[:])
            nc.sync.dma_start(out_colsum[:], csacc[:])

    nc.compile()
    return nc


def get_program():
    if "nc" not in _CACHE:
        _CACHE["nc"] = _build()
    return _CACHE["nc"]


def _quad_pack(a):  # [R, 512] -> [128, R//128, 512], rows r = s*128+p
    return np.ascontiguousarray(a.reshape(-1, 128, D).transpose(1, 0, 2))


def make_in_maps(z_img, z_text):
    zi = np.asarray(z_img, dtype=np.float32).astype(ml_dtypes.bfloat16)
    zt = np.asarray(z_text, dtype=np.float32).astype(ml_dtypes.bfloat16)
    ztT = np.ascontiguousarray(zt.T)
    ztq = np.ascontiguousarray(
        zt.reshape(NO, 8, 128, D).transpose(0, 2, 1, 3))
    maps = []
    for c in range(CORES):
        blk = slice(c * BLK, (c + 1) * BLK)
        maps.append({
            "z_img_q": _quad_pack(zi[blk]),
            "z_textT": ztT,
            "z_text_q": ztq,
            "z_blk_q": _quad_pack(zt[blk]),
            "ident": np.eye(128).astype(ml_dtypes.bfloat16),
        })
    return maps


def combine(results):
    rows = np.concatenate([r["out_rowsum"][0] for r in results])          # [8192]
    cols = np.zeros((128, NT), np.float64)
    for r in results:
        cols += r["out_colsum"]
    colsum = cols.T.reshape(-1)                                           # j = jt*128+p
    diag = np.concatenate([r["out_diag"].T.reshape(-1) for r in results])
    loss_a = np.mean(np.log(rows.astype(np.float64)) - diag)
    loss_b = np.mean(np.log(colsum) - diag)
    return np.float32(ALPHA * loss_a + (1.0 - ALPHA) * loss_b)


def _run_sim(nc, maps):
    from concourse.bass_interp import CoreSim
    outs = []
    for m in maps:
        sim = CoreSim(nc, trace=False)
        for k, v in m.items():
            sim.tensor(k)[:] = v
        sim.simulate()
        outs.append({n: np.array(sim.tensor(n))
                     for n in ("out_rowsum", "out_colsum", "out_diag")})
    return outs


def kernel(z_img, z_text):
    nc = get_program()
    maps = make_in_maps(z_img, z_text)
    for _ in range(3):
        try:
            res = run_bass_kernel_spmd(nc, maps, list(range(CORES))).results
            return combine(res)
        except Exception:  # transient device hiccups: retry, then sim
            pass
    res = _run_sim(nc, maps)
    return combine(res)


if __name__ == "__main__":
    rng = np.random.default_rng(0)
    out = kernel(rng.standard_normal((N, D), dtype=np.float32),
                 rng.standard_normal((N, D), dtype=np.float32))
    print("loss:", out)


# revision 10
# speedup vs baseline: 1.4790x; 1.0525x over previous
"""ConVIRT loss (NT-Xent both directions) on 8 Trainium2 NeuronCores.

Strategy: shard img rows across 8 cores; each core computes its
[8192 (text j) x 1024 (img i)] slab of the similarity matrix in fp8
DoubleRow matmuls (2x PE throughput).  The img operand carries its
cosine norm (x32 fp8 range scale); the text norm rides the per-partition
ACT exp scale.

Feeds per core (host-prepped, bf16; packing is pure layout):
  z_img_q  [128, 8, 512]     img block rows slot-packed (one DMA)
  z_textT  [512, 8192]       full text TRANSPOSED -> fp8 matmul operand
                             via gpsimd DMA-cast (no device transposes)
  z_text_q [8, 128, 8, 512]  full text rows oct-packed (row sumsq)
  z_blk_q  [128, 8, 512]     core's text block rows (diag + norms)

Main loop per jt: 4 fp8 DR matmuls fill a 2-bank psum tile [j=128,
i=1024]; ONE ACT exp (N=1024 amortizes the 352-cycle ACT fixed cost)
writes an fp8 e-tile slot and accumulates colsum; per jt-pair two fp8
DR ones-matmuls accumulate rowsum in psum.  All norms (1/sqrt) are
computed on DVE with the bit-trick + 2 Newton steps so the ACT table
set never leaves Exp (table reloads cost 1.3us each).  Input DMAs ride
the sync-engine HWDGE queue; fp8 casts ride the gpsimd swdge.
Host combines: loss = a*mean(log(rs)-d) + (1-a)*mean(log(cs)-d).
"""

import math
import numpy as np
import ml_dtypes

import concourse.bacc as bacc
import concourse.tile as tile
import concourse.mybir as mybir
from concourse.bass_utils import run_bass_kernel_spmd

N, D = 8192, 512
CORES = 8
BLK = N // CORES          # 1024 img rows per core
NT = N // 128             # 64 text j-tiles
NTI = BLK // 128          # 8 img tiles per core
KC = D // 128             # 4 contraction chunks
KP = KC // 2              # 2 fp8 DoubleRow chunk-pairs
IC = BLK // 512           # 2 rowsum chunks of 512
NPAIR = NT // 2           # 32 jt-pairs for the rowsum ones-matmul
NO = 8                    # text row octs
TPC = 2048                # textT DMA-cast piece columns
FS = 32.0                 # fp8 range scale on the img operand
TEMP, ALPHA = 0.1, 0.75

f32 = mybir.dt.float32
bf16 = mybir.dt.bfloat16
fp8 = mybir.dt.float8e4
i32 = mybir.dt.int32
AF = mybir.ActivationFunctionType
ALU = mybir.AluOpType
PM = mybir.MatmulPerfMode

_CACHE = {}


def _rsqrt(nc, pool, ss_ap, out_ap, mult):
    """out = mult / sqrt(ss) on DVE: Quake bit-trick + 2 Newton steps.

    Avoids ACT Ln/Sqrt so the exp table set stays resident.  ~1e-5 rel.
    """
    n = ss_ap.shape[-1]
    y = pool.tile([128, n], f32, tag="rq_y")
    t = pool.tile([128, n], f32, tag="rq_t")
    # y = bitcast(0x5f3759df - (bitcast_i32(ss) >> 1)) = ~((ss>>1)^-1)+C+1
    nc.vector.tensor_scalar(y[:].bitcast(i32), ss_ap.bitcast(i32),
                            1, -1, ALU.logical_shift_right,
                            ALU.bitwise_xor)
    nc.vector.tensor_scalar_add(y[:].bitcast(i32), y[:].bitcast(i32),
                                0x5F3759E0)
    # Newton 1: y *= 1.5 - 0.5*ss*y^2
    nc.vector.tensor_tensor(t[:], y[:], y[:], op=ALU.mult)
    nc.vector.tensor_tensor(t[:], t[:], ss_ap, op=ALU.mult)
    nc.vector.tensor_scalar(t[:], t[:], -0.5, 1.5, ALU.mult, ALU.add)
    nc.vector.tensor_tensor(y[:], y[:], t[:], op=ALU.mult)
    # Newton 2, final mult folded in: y *= c*1.5 - c*0.5*ss*y^2
    nc.vector.tensor_tensor(t[:], y[:], y[:], op=ALU.mult)
    nc.vector.tensor_tensor(t[:], t[:], ss_ap, op=ALU.mult)
    nc.vector.tensor_scalar(t[:], t[:], -0.5 * mult, 1.5 * mult,
                            ALU.mult, ALU.add)
    nc.vector.tensor_tensor(out_ap, y[:], t[:], op=ALU.mult)


def _build():
    nc = bacc.Bacc("TRN2", target_bir_lowering=False, debug=False)

    z_img_q = nc.dram_tensor("z_img_q", [128, NTI, D], bf16, kind="ExternalInput")
    z_textT = nc.dram_tensor("z_textT", [D, N], bf16, kind="ExternalInput")
    z_text_q = nc.dram_tensor("z_text_q", [NO, 128, 8, D], bf16, kind="ExternalInput")
    z_blk_q = nc.dram_tensor("z_blk_q", [128, NTI, D], bf16, kind="ExternalInput")
    ident = nc.dram_tensor("ident", [128, 128], bf16, kind="ExternalInput")
    out_rowsum = nc.dram_tensor("out_rowsum", [1, BLK], f32, kind="ExternalOutput")
    out_colsum = nc.dram_tensor("out_colsum", [128, NT], f32, kind="ExternalOutput")
    out_diag = nc.dram_tensor("out_diag", [128, NTI], f32, kind="ExternalOutput")

    with tile.TileContext(nc) as tc:
        with (
            tc.tile_pool(name="pers", bufs=1) as pers,
            tc.tile_pool(name="ldq", bufs=3) as ldqpool,
            tc.tile_pool(name="sq", bufs=3) as sqpool,
            tc.tile_pool(name="rq", bufs=2) as rqpool,
            tc.tile_pool(name="e2", bufs=3) as e2pool,
            tc.tile_pool(name="ps", bufs=2, space="PSUM") as pspool,
            tc.tile_pool(name="psr", bufs=1, space="PSUM") as psrpool,
            tc.tile_pool(name="pst", bufs=2, space="PSUM") as pstpool,
        ):
            imgq = pers.tile([128, NTI, D], bf16, tag="imgq")
            identSB = pers.tile([128, 128], bf16, tag="identSB")
            tblk = pers.tile([128, NTI, D], bf16, tag="tblk")
            textT8 = [pers.tile([128, 2, N], fp8, tag=f"textT8_{p}", name=f"textT8_{p}")
                      for p in range(KP)]
            imgT8 = [pers.tile([128, 2, BLK], fp8, tag=f"imgT8_{p}", name=f"imgT8_{p}")
                     for p in range(KP)]
            img_nb = pers.tile([128, NTI, D], bf16, tag="img_nb")
            tss = pers.tile([128, NT], f32, tag="tss")
            tscale_q = [pers.tile([128, 8], f32, tag=f"tscale{q}", name=f"tscale{q}")
                        for q in range(NO)]
            iss = pers.tile([128, NTI], f32, tag="iss")
            iscale = pers.tile([128, NTI], f32, tag="iscale")
            bss = pers.tile([128, NTI], f32, tag="bss")
            bscale = pers.tile([128, NTI], f32, tag="bscale")
            dots = pers.tile([128, NTI], f32, tag="dots")
            diagb = pers.tile([128, NTI], f32, tag="diagb")
            csacc = pers.tile([128, NT], f32, tag="csacc")
            rs = pers.tile([1, BLK], f32, tag="rs")

            # DR weights need the 2-slot dim stride %16 B: pad cols to 16
            ones8 = pers.tile([128, 2, 16], fp8, tag="ones8")
            nc.vector.memset(ones8[:], 1.0)
            # warm up the Exp table set immediately (load costs 1.3us and
            # otherwise lands on the first real exp's critical path)
            wrm = pers.tile([128, 1], f32, tag="wrm")
            nc.vector.memset(wrm[:], 1.0)
            nc.scalar.activation(wrm[:], wrm[:], AF.Exp)
            gate = pers.tile([1, 16], bf16, tag="gate")

            # ---- input DMA triggers in need-order.
            # sync HWDGE: img gates PE; oct0 gates ACT's first exp scale.
            nc.sync.dma_start(imgq[:], z_img_q[:])
            octs = []
            for q in range(NO):
                qt = ldqpool.tile([128, 8, D], bf16, tag="ldq", name=f"oct{q}")
                nc.sync.dma_start(qt[:], z_text_q[q])
                octs.append(qt)
                if q == 0:
                    nc.sync.dma_start(identSB[:], ident[:])
                    nc.sync.dma_start(tblk[:], z_blk_q[:])
            # gpsimd swdge fp8 DMA-casts, first piece (jt 0..15) first.
            # The gate copy stalls the cast stream until the img DMA lands,
            # so the (PE-gating) img feed isn't starved by cast traffic.
            nc.gpsimd.tensor_copy(gate[:], imgq[0:1, 0, 0:16])
            for pc in range(N // TPC):
                cs = slice(pc * TPC, (pc + 1) * TPC)
                for k in range(KC):
                    nc.gpsimd.dma_start(textT8[k // 2][:, k % 2, cs],
                                        z_textT[k * 128:(k + 1) * 128, cs])

            # ---- img: sumsq, normalize (FS/r_i folded, bf16), transpose.
            # high_priority pins this chain ahead of the oct sumsq stream on
            # DVE -- it gates the PE main loop.
            with tc.high_priority():
                for t in range(NTI):
                    sq = sqpool.tile([128, D], bf16, tag="sq")
                    nc.vector.affine_mul_reduce(
                        sq[:], iss[:, t:t + 1], imgq[:, t, :], imgq[:, t, :], 1.0, 0.0)
                _rsqrt(nc, rqpool, iss[:], iscale[:], FS)
                for t in range(NTI):
                    nc.vector.tensor_scalar(
                        img_nb[:, t, :], imgq[:, t, :], iscale[:, t:t + 1], None, ALU.mult)
                    for p in range(KP):
                        pst = pstpool.tile([128, 2, 128], bf16, tag="pst")
                        for c in range(2):
                            nc.tensor.transpose(
                                pst[:, c, :],
                                img_nb[:, t, (2 * p + c) * 128:(2 * p + c + 1) * 128],
                                identSB[:])
                        nc.vector.tensor_copy(imgT8[p][:, :, t * 128:(t + 1) * 128], pst[:])

            # ---- text row sumsq -> tscale, block sumsq, diag dots.
            # Deprioritized (negative offset) so the static DVE schedule runs
            # the (PE-gating) img chain first; RAW deps still hold since this
            # is emitted before the main loop.
            with tc.high_priority(offset=-1000000):
                for q in range(NO):
                    for s in range(8):
                        sq = sqpool.tile([128, D], bf16, tag="sq")
                        nc.vector.affine_mul_reduce(
                            sq[:], tss[:, 8 * q + s:8 * q + s + 1],
                            octs[q][:, s, :], octs[q][:, s, :], 1.0, 0.0)
                    _rsqrt(nc, rqpool, tss[:, 8 * q:8 * q + 8],
                           tscale_q[q][:], 1.0 / (FS * TEMP))
                for t in range(NTI):
                    sq = sqpool.tile([128, D], bf16, tag="sq")
                    nc.vector.scalar_tensor_tensor(
                        out=sq[:], in0=tblk[:, t, :], scalar=1.0, in1=tblk[:, t, :],
                        op0=ALU.mult, op1=ALU.mult, accum_out=bss[:, t:t + 1])
                _rsqrt(nc, rqpool, bss[:], bscale[:], 1.0 / (FS * TEMP))
                # diag_r = dot(FS*img_n[r], text_raw[r]) / (FS*T*t_r) = cos/T
                for t in range(NTI):
                    sq = sqpool.tile([128, D], bf16, tag="sq")
                    nc.vector.affine_mul_reduce(
                        sq[:], dots[:, t:t + 1], img_nb[:, t, :], tblk[:, t, :], 1.0, 0.0)
                nc.vector.tensor_tensor(diagb[:], dots[:], bscale[:], op=ALU.mult)

            # ---- main loop: 2-bank psum tile [j=128, i=1024] per jt.
            # The rowsum ones-matmuls are software-pipelined one jt-pair
            # behind the sim matmuls: they consume EXP output, and emitting
            # them in-order would stall PE on ACT every pair.
            # Colsum: even jt via ACT accum (exact), odd jt via DVE reduce
            # of the fp8 e-tile -- halves the ACT read-accumulator tax.
            psrow = [psrpool.tile([1, 512], f32, tag=f"psr{ic}", name=f"psr{ic}")
                     for ic in range(IC)]

            def ones_mm(pr, e2prev):
                for ic in range(IC):
                    nc.tensor.matmul(
                        psrow[ic][:], ones8[:, :, 0:1],
                        e2prev[:, :, ic * 512:(ic + 1) * 512],
                        start=(pr == 0), stop=(pr == NPAIR - 1),
                        perf_mode=PM.DoubleRow,
                        skip_group_check=True)

            e2t, e2prev = None, None
            for jt in range(NT):
                pr, slot = jt // 2, jt % 2
                if slot == 0:
                    e2t = e2pool.tile([128, 2, 1024], fp8, tag="e2", name=f"e2_{pr}")
                ps = pspool.tile([128, 1024], f32, tag="ps")
                for ic in range(IC):
                    for p in range(KP):
                        nc.tensor.matmul(
                            ps[:, ic * 512:(ic + 1) * 512],
                            textT8[p][:, :, jt * 128:(jt + 1) * 128],
                            imgT8[p][:, :, ic * 512:(ic + 1) * 512],
                            start=(p == 0), stop=(p == KP - 1),
                            perf_mode=PM.DoubleRow)
                if slot == 0:
                    nc.scalar.activation(
                        e2t[:, 0, :], ps[:], AF.Exp,
                        scale=tscale_q[jt // 8][:, jt % 8:jt % 8 + 1],
                        accum_out=csacc[:, jt:jt + 1])
                else:
                    nc.scalar.activation(
                        e2t[:, 1, :], ps[:], AF.Exp,
                        scale=tscale_q[jt // 8][:, jt % 8:jt % 8 + 1])
                    nc.vector.tensor_reduce(
                        csacc[:, jt:jt + 1], e2t[:, 1, :],
                        axis=mybir.AxisListType.X, op=ALU.add)
                    if pr >= 1:
                        ones_mm(pr - 1, e2prev)
                    e2prev = e2t
            ones_mm(NPAIR - 1, e2prev)


            # ---- finish: rowsum psum -> sbuf -> dram; colsum/diag direct
            for ic in range(IC):
                nc.vector.tensor_copy(rs[:, ic * 512:(ic + 1) * 512], psrow[ic][:])
            nc.sync.dma_start(out_diag[:], diagb[:])
            nc.sync.dma_start(out_rowsum[:], rs[:])
            nc.sync.dma_start(out_colsum[:], csacc[:])

    nc.compile()
    return nc


def get_program():
    if "nc" not in _CACHE:
        _CACHE["nc"] = _build()
    return _CACHE["nc"]


def _quad_pack(a):  # [R, 512] -> [128, R//128, 512], rows r = s*128+p
    return np.ascontiguousarray(a.reshape(-1, 128, D).transpose(1, 0, 2))


def make_in_maps(z_img, z_text):
    zi = np.asarray(z_img, dtype=np.float32).astype(ml_dtypes.bfloat16)
    zt = np.asarray(z_text, dtype=np.float32).astype(ml_dtypes.bfloat16)
    ztT = np.ascontiguousarray(zt.T)
    ztq = np.ascontiguousarray(
        zt.reshape(NO, 8, 128, D).transpose(0, 2, 1, 3))
    maps = []
    for c in range(CORES):
        blk = slice(c * BLK, (c + 1) * BLK)
        maps.append({
            "z_img_q": _quad_pack(zi[blk]),
            "z_textT": ztT,
            "z_text_q": ztq,
            "z_blk_q": _quad_pack(zt[blk]),
            "ident": np.eye(128).astype(ml_dtypes.bfloat16),
        })
    return maps


def combine(results):
    rows = np.concatenate([r["out_rowsum"][0] for r in results])          # [8192]
    cols = np.zeros((128, NT), np.float64)
    for r in results:
        cols += r["out_colsum"]
    colsum = cols.T.reshape(-1)                                           # j = jt*128+p
    diag = np.concatenate([r["out_diag"].T.reshape(-1) for r in results])
    loss_a = np.mean(np.log(rows.astype(np.float64)) - diag)
    loss_b = np.mean(np.log(colsum) - diag)
    return np.float32(ALPHA * loss_a + (1.0 - ALPHA) * loss_b)


def _run_sim(nc, maps):
    from concourse.bass_interp import CoreSim
    outs = []
    for m in maps:
        sim = CoreSim(nc, trace=False)
        for k, v in m.items():
            sim.tensor(k)[:] = v
        sim.simulate()
        outs.append({n: np.array(sim.tensor(n))
                     for n in ("out_rowsum", "out_colsum", "out_diag")})
    return outs


def kernel(z_img, z_text):
    nc = get_program()
    maps = make_in_maps(z_img, z_text)
    for _ in range(3):
        try:
            res = run_bass_kernel_spmd(nc, maps, list(range(CORES))).results
            return combine(res)
        except Exception:  # transient device hiccups: retry, then sim
            pass
    res = _run_sim(nc, maps)
    return combine(res)


if __name__ == "__main__":
    rng = np.random.default_rng(0)
    out = kernel(rng.standard_normal((N, D), dtype=np.float32),
                 rng.standard_normal((N, D), dtype=np.float32))
    print("loss:", out)
